# revision 1
# baseline (speedup 1.0000x reference)
"""Trainium2 Bass kernel for the Krylov/SSM problem.

K[h, l] = C[h] @ dA[h]^l @ dB[h],  l = 0..L-1
  dA = (I - (dt/2)A)^-1 (I + (dt/2)A),  dB = dt * (I - (dt/2)A)^-1 B

Device algorithm (per channel, fp32 throughout):
  E = (dt/2) A           (host prep, packed 2 channels per 128x128 block-diag tile)
  Neumann-product inverse: G = (I+E)(I+E^2)(I+E^4)(I+E^8)   [rho(E) <= ~0.2]
  dA  = (I+E)^2 (I+E^2)(I+E^4)(I+E^8)
  Ct  = G^T C            (solve folded into the C side; G commutes with dA)
  V   = [dt*B, dA(dt*B), ..., dA^63(dt*B)]        (doubling)
  U   = [Ct, M^T Ct, ..., (M^T)^63 Ct], M = dA^64 (doubling, powers to dA^2048)
  K[h, 64j + i] = (U^T V)[j, i]

All matmuls run as 128x128 block-diagonal ops (2 channels / PE pass). The
matmul primitive computes lhsT.T @ rhs, so the squaring chain keeps both
X and X^T for every power ("pair squaring"); each level's (N, T) pair lands
in one PSUM bank and moves to SBUF with a single [128,256] copy.

Instructions are emitted LEVEL-MAJOR across the 16 per-core tiles so each
engine's (in-order) instruction stream always has ready work from other
tiles while one tile's dependencies drain.

Sharding: H axis, 32 channels (16 tiles) per core across 8 cores. SPMD, no
communication.
"""

import numpy as np

H, N, L = 256, 64, 4096
NCORES = 8
CH_PER_CORE = H // NCORES   # 32
NT = CH_PER_CORE // 2       # 16 block-diag tiles per core

_cache = {}


def _build_program(nt=NT, repeat=None):
    import contextlib
    import concourse.bacc as bacc
    import concourse.tile as tile
    import concourse.mybir as mybir

    f32 = mybir.dt.float32
    nc = bacc.Bacc("TRN2", target_bir_lowering=False, debug=False)

    Ein = nc.dram_tensor("Ein", [nt, 128, 128], f32, kind="ExternalInput").ap()
    ETin = nc.dram_tensor("ETin", [nt, 128, 128], f32, kind="ExternalInput").ap()
    H0in = nc.dram_tensor("H0in", [nt, 128, 128], f32, kind="ExternalInput").ap()
    H0Tin = nc.dram_tensor("H0Tin", [nt, 128, 128], f32, kind="ExternalInput").ap()
    BCin = nc.dram_tensor("BCin", [nt, 128, 2], f32, kind="ExternalInput").ap()
    Iin = nc.dram_tensor("Iin", [128, 128], f32, kind="ExternalInput").ap()
    Y = nc.dram_tensor("Y", [nt, 128, 64], f32, kind="ExternalOutput").ap()

    with tile.TileContext(nc) as tc:
        with (
            tc.tile_pool(name="const", bufs=1) as cpool,
            tc.tile_pool(name="sb", bufs=1) as sb,
            tc.tile_pool(name="ps", bufs=1, space="PSUM") as ps,
        ):
            ident = cpool.tile([128, 128], f32, name="ident")
            nc.sync.dma_start(ident[:], Iin)
            rep = tc.For_i(0, repeat, 1) if repeat else contextlib.nullcontext()
            rep.__enter__()

            state = {"i": 0}

            def ve():  # 2:1 DVE:ACT split for PSUM->SBUF traffic
                state["i"] += 1
                return state["i"] % 9 < 5

            def SBT(tag, w=128, bufs=nt + 1):
                return sb.tile([128, w], f32, tag=tag, name=tag, bufs=bufs)

            def PW(w=256):
                """One PSUM bank (pair = [N | T] side by side when w=256)."""
                return ps.tile([128, w], f32, tag="mm", bufs=6, name="pw")

            def MM(out_ap, lhsT, rhs):
                nc.tensor.matmul(out_ap, lhsT, rhs, start=True, stop=True)

            def CPW(tag, p, w=256, bufs=None):
                # all wide stage tensors share one ring: ~2 levels x nt live
                s = sb.tile([128, w], f32, tag="ring", name=tag,
                            bufs=2 * nt + 4)
                if ve():
                    nc.vector.tensor_copy(s[:], p[:, 0:w])
                else:
                    nc.scalar.copy(s[:], p[:, 0:w])
                return s

            def SMM(lhsT, rhs, w):
                p = ps.tile([128, w], f32, tag="sm", bufs=2, name="sp")
                nc.tensor.matmul(p[:], lhsT, rhs, start=True, stop=True)
                return p

            T = [dict() for _ in range(nt)]
            tiles = range(nt)

            def s_load(t):
                d = T[t]
                dma = [nc.sync, nc.gpsimd][t % 2]
                for nm, srcap in (("E", Ein), ("ET", ETin), ("H0", H0in),
                                  ("H0T", H0Tin)):
                    d[nm] = SBT(nm, bufs=8)
                    dma.dma_start(d[nm][:], srcap[t])
                d["BC"] = SBT("BC", w=2)
                dma.dma_start(d["BC"][:], BCin[t])

            def s_f0(t):
                d = T[t]
                d["F0"] = SBT("F0", bufs=12)
                nc.gpsimd.tensor_add(d["F0"][:], d["E"][:], ident[:])
                d["F0T"] = SBT("F0T", bufs=8)
                nc.gpsimd.tensor_add(d["F0T"][:], d["ET"][:], ident[:])

            def s_e2(t):
                d = T[t]
                p = PW()
                MM(p[:, 0:128], d["ET"][:], d["E"][:])
                MM(p[:, 128:256], d["E"][:], d["ET"][:])
                d["E2NT"] = CPW("E2NT", p)
                d["F1"] = SBT("F1", bufs=12)
                nc.gpsimd.tensor_add(d["F1"][:], d["E2NT"][:, 0:128], ident[:])
                d["F0sq"] = SBT("F0sq", bufs=8)
                nc.vector.tensor_add(d["F0sq"][:], p[:, 0:128], d["H0"][:])
                d["F0sqT"] = SBT("F0sqT", bufs=8)
                nc.vector.tensor_add(d["F0sqT"][:], p[:, 128:256], d["H0T"][:])

            def s_e4(t):
                d = T[t]
                p = PW()
                MM(p[:, 0:128], d["E2NT"][:, 128:256], d["E2NT"][:, 0:128])
                MM(p[:, 128:256], d["E2NT"][:, 0:128], d["E2NT"][:, 128:256])
                d["E4NT"] = CPW("E4NT", p)
                d["F2"] = SBT("F2", bufs=12)
                nc.gpsimd.tensor_add(d["F2"][:], d["E4NT"][:, 0:128], ident[:])
                d["F2T"] = SBT("F2T", bufs=8)
                nc.gpsimd.tensor_add(d["F2T"][:], d["E4NT"][:, 128:256], ident[:])

            def s_e8(t):
                d = T[t]
                p = SMM(d["E4NT"][:, 128:256], d["E4NT"][:, 0:128], w=128)
                d["F3"] = SBT("F3", bufs=12)
                nc.vector.tensor_add(d["F3"][:], p[:], ident[:])

            def s_pp(t):
                d = T[t]
                p = PW()
                MM(p[:, 0:128], d["F1"][:], d["F0sqT"][:])
                MM(p[:, 128:256], d["F2T"][:], d["F3"][:])
                d["PP"] = CPW("PP", p)

            def s_nt0(t):
                d = T[t]
                PAT, PB = d["PP"][:, 0:128], d["PP"][:, 128:256]
                p = PW()
                MM(p[:, 0:128], PAT, PB)
                MM(p[:, 128:256], PB, PAT)
                d["NT0"] = CPW("NT0", p)

            def mk_c(ci):
                def s_c(t):
                    d = T[t]
                    if ci == 0:
                        d["U"] = SBT("U", w=64, bufs=nt + 1)
                        d["V"] = SBT("V", w=64, bufs=nt + 1)
                        nc.scalar.copy(d["V"][:, 0:1], d["BC"][:, 0:1])
                    Fk = d[("F0", "F1", "F2", "F3")[ci]]
                    c = d["BC"][:, 1:2] if ci == 0 else d[f"c{ci-1}"][:]
                    cp = SMM(Fk[:], c, w=1)
                    if ci < 3:
                        cs = SBT(f"c{ci}", w=1, bufs=8)
                        nc.scalar.copy(cs[:], cp[:])
                        d[f"c{ci}"] = cs
                    else:
                        nc.scalar.copy(d["U"][:, 0:1], cp[:])
                return s_c

            def mk_pow(k):
                def s_pow(t):
                    d = T[t]
                    if k <= 11:
                        Nk1 = d[f"NT{k-1}"][:, 0:128]
                        Tk1 = d[f"NT{k-1}"][:, 128:256]
                    if k <= 6:
                        w = 1 << (k - 1)
                        vp = SMM(Tk1, d["V"][:, 0:w], w=w)
                        if ve():
                            nc.vector.tensor_copy(d["V"][:, w:2 * w], vp[:])
                        else:
                            nc.scalar.copy(d["V"][:, w:2 * w], vp[:])
                    if k >= 7:
                        j = k - 7          # U-apply j reads NT_{6+j} = NT_{k-1}
                        w = 1 << j
                        up = SMM(d[f"NT{6+j}"][:, 0:128], d["U"][:, 0:w], w=w)
                        if ve():
                            nc.vector.tensor_copy(d["U"][:, w:2 * w], up[:])
                        else:
                            nc.scalar.copy(d["U"][:, w:2 * w], up[:])
                    if k <= 10:
                        p = PW()
                        MM(p[:, 0:128], Tk1, Nk1)
                        MM(p[:, 128:256], Nk1, Tk1)
                        d[f"NT{k}"] = CPW(f"NT{k}", p)
                    elif k == 11:
                        p = SMM(Tk1, Nk1, w=128)
                        d[f"NT{k}"] = CPW(f"NT{k}", p, w=128)
                return s_pow

            def s_fin(t):
                d = T[t]
                Kap = ps.tile([64, 64], f32, tag="sm", bufs=2, name="Kap")
                nc.tensor.matmul(Kap[:], d["U"][0:64, :], d["V"][0:64, :],
                                 start=True, stop=True)
                Ka = sb.tile([64, 64], f32, tag="Ka", name="Ka", bufs=4)
                nc.scalar.copy(Ka[:], Kap[:])
                nc.scalar.dma_start(Y[t, 0:64, :], Ka[:])
                Kbp = ps.tile([64, 64], f32, tag="sm", bufs=2, name="Kbp")
                nc.tensor.matmul(Kbp[:], d["U"][64:128, :], d["V"][64:128, :],
                                 start=True, stop=True)
                Kb = sb.tile([64, 64], f32, tag="Kb", name="Kb", bufs=4)
                nc.vector.tensor_copy(Kb[:], Kbp[:])
                nc.sync.dma_start(Y[t, 64:128, :], Kb[:])

            stages = ([s_load, s_f0, s_e2, s_e4, s_e8] +
                      [mk_c(0), mk_c(1), mk_c(2), mk_c(3)] +
                      [s_pp, s_nt0] +
                      [mk_pow(k) for k in range(1, 13)] +
                      [s_fin])
            ns = len(stages)
            # skewed (wavefront) emission: tile t runs stage s at step s + t
            for step in range(ns + nt - 1):
                for t in tiles:
                    s = step - t
                    if 0 <= s < ns:
                        stages[s](t)
            rep.__exit__(None, None, None)

    nc.compile()
    return nc


def _host_pack(A, B, C, log_dt):
    A = np.asarray(A, np.float32)
    B = np.asarray(B, np.float32)
    C = np.asarray(C, np.float32)
    log_dt = np.asarray(log_dt, np.float32)
    dt = np.exp(log_dt)
    E = (0.5 * dt)[:, None, None].astype(np.float32) * A      # [H,64,64]
    ETc = np.ascontiguousarray(np.swapaxes(E, 1, 2))
    dtB = (dt[:, None] * B).astype(np.float32)

    G = A.shape[0] // 2
    Epk = np.zeros((G, 128, 128), np.float32)
    ETpk = np.zeros((G, 128, 128), np.float32)
    Epk[:, 0:64, 0:64] = E[0::2]
    Epk[:, 64:128, 64:128] = E[1::2]
    ETpk[:, 0:64, 0:64] = ETc[0::2]
    ETpk[:, 64:128, 64:128] = ETc[1::2]
    I128 = np.eye(128, dtype=np.float32)
    H0pk = 2.0 * Epk + I128
    H0Tpk = 2.0 * ETpk + I128
    BCpk = np.zeros((G, 128, 2), np.float32)
    BCpk[:, 0:64, 0] = dtB[0::2]
    BCpk[:, 64:128, 0] = dtB[1::2]
    BCpk[:, 0:64, 1] = C[0::2]
    BCpk[:, 64:128, 1] = C[1::2]
    return Epk, ETpk, H0pk, H0Tpk, BCpk, I128


def _in_maps(A, B, C, log_dt):
    Epk, ETpk, H0pk, H0Tpk, BCpk, I128 = _host_pack(A, B, C, log_dt)
    maps = []
    for c in range(NCORES):
        s = slice(c * NT, (c + 1) * NT)
        maps.append({"Ein": Epk[s], "ETin": ETpk[s], "H0in": H0pk[s],
                     "H0Tin": H0Tpk[s], "BCin": BCpk[s], "Iin": I128})
    return maps


def kernel(A, B, C, log_dt, L):
    from concourse.bass_utils import run_bass_kernel_spmd

    if "nc" not in _cache:
        _cache["nc"] = _build_program(NT)
    nc = _cache["nc"]

    res = run_bass_kernel_spmd(nc, _in_maps(A, B, C, log_dt),
                               core_ids=list(range(NCORES)))
    K = np.empty((H, L), np.float32)
    for c in range(NCORES):
        K[c * CH_PER_CORE:(c + 1) * CH_PER_CORE] = (
            res.results[c]["Y"].reshape(CH_PER_CORE, L))
    return K



# revision 15
# speedup vs baseline: 1.9383x; 1.9383x over previous
"""Trainium2 Bass kernel for the Krylov/SSM problem.

K[h, l] = C[h] @ dA[h]^l @ dB[h],  l = 0..L-1
  dA = (I - (dt/2)A)^-1 (I + (dt/2)A),  dB = dt * (I - (dt/2)A)^-1 B

Device algorithm (per channel):
  E = (dt/2) A           (host prep, packed 2 channels per 128x128 block-diag tile)
  Neumann-product inverse: G = (I+E)(I+E^2)(I+E^4)(I+E^8)   [rho(E) <= ~0.25]
  dA  = (I+E)^2 (I+E^2)(I+E^4)(I+E^8)
  Ct  = G^T C            (solve folded into the C side; G commutes with dA)
  V   = [dt*B, dA(dt*B), ..., dA^63(dt*B)]        (doubling)
  U   = [Ct, M^T Ct, ..., (M^T)^63 Ct], M = dA^64 (doubling, powers to dA^2048)
  K[h, 64j + i] = (U^T V)[j, i]

All data is fp16 (PSUM accumulation is fp32); numpy simulation of the full
fp16 pipeline gives rel err ~8e-3 vs the fp32 reference (tolerance 2e-2).
fp16 matmuls stream at 1 cycle/row on the PE (4x faster than fp32) and
128-col fp16 stationaries get fast-weight-load.

The matmul primitive computes lhsT.T @ rhs, so the squaring chain keeps both
X and X^T for every power ("pair squaring").  The V/U doubling applies are
FOLDED into the chain matmuls: the apply at level k uses the same stationary
operand as the squaring (T_{k-1} for V / N_{k-1} for U), so the moving
operand is just widened to [N_{k-1} | V-cols] / [T_{k-1} | U-cols].  V/U
columns therefore live INSIDE the ring tiles, appended newest-first (the
final column order is fully reversed; the host un-reverses when unpacking).

Ring layout [128, 2(side N/T), 2(tile), 192]: per tile, side 0 = [N | Vcols],
side 1 = [T | Ucols].  Each level's products land in one two-bank PSUM tile
([128, 2(side), 2(tile), 256]) evacuated by a single 4D strided copy; the
old V/U columns are carried forward by a small Pool-engine copy.

Tiles are processed in PAIRS (the two tiles of a pair share PSUM tiles and
wide evacuations).  Instructions are emitted in a skewed wavefront across
the 16 per-core tiles, descending tile order within a step so pair-shared
tensors produced by the odd tile precede the even tile's next stage.

Sharding: H axis, 32 channels (16 tiles) per core across 8 cores. SPMD, no
communication.
"""

import numpy as np

H, N, L = 256, 64, 4096
NCORES = 8
CH_PER_CORE = H // NCORES   # 32
NT = CH_PER_CORE // 2       # 16 block-diag tiles per core
NP = NT // 2                # 8 tile pairs

_cache = {}


def _build_program(nt=NT, repeat=None):
    import contextlib
    import concourse.bacc as bacc
    import concourse.tile as tile
    import concourse.mybir as mybir

    f16 = mybir.dt.float16
    f32 = mybir.dt.float32
    npair = nt // 2
    nc = bacc.Bacc("TRN2", target_bir_lowering=False, debug=False)

    # IN cols per tile: [E | ET | H0 | H0T], fp16
    INd = nc.dram_tensor("INd", [128, nt * 512], f16, kind="ExternalInput").ap()
    # BC cols per pair: [dtB_a | dtB_b | C_a | C_b]
    BCd = nc.dram_tensor("BCd", [128, nt * 2], f16, kind="ExternalInput").ap()
    IWd = nc.dram_tensor("IWd", [128, 256], f16, kind="ExternalInput").ap()
    Y = nc.dram_tensor("Y", [64, nt * 128], f32, kind="ExternalOutput").ap()

    with tile.TileContext(nc) as tc:
        with (
            tc.tile_pool(name="const", bufs=1) as cpool,
            tc.tile_pool(name="sb", bufs=1) as sb,
            tc.tile_pool(name="ps", bufs=1, space="PSUM") as ps,
        ):
            IN = cpool.tile([128, nt, 512], f16, name="IN")
            BC = cpool.tile([128, nt * 2], f16, name="BC")
            IW = cpool.tile([128, 2, 128], f16, name="IW")
            OUT = cpool.tile([64, nt * 128], f32, name="OUT")
            nc.sync.dma_start(IW[:], IWd)
            nc.sync.dma_start(BC[:], BCd)

            rep = tc.For_i(0, repeat, 1) if repeat else contextlib.nullcontext()
            rep.__enter__()

            # input DMAs: 4 chunks of 4 tiles on the 3 DMA-capable queues
            chunk = nt // 4
            for ci, eng in enumerate((nc.sync, nc.gpsimd, nc.scalar,
                                      nc.gpsimd)):
                eng.dma_start(IN[:, ci * chunk:(ci + 1) * chunk, :],
                              INd[:, ci * chunk * 512:(ci + 1) * chunk * 512])

            state = {"i": 0, "j": 0}

            def _cp(out, in_, dve):
                if dve:
                    nc.vector.tensor_copy(out, in_)
                else:
                    nc.scalar.copy(out, in_)

            def ev(out, in_):  # evac copy, engine-rotated (DVE : ACT)
                state["i"] += 1
                _cp(out, in_, state["i"] % 2)

            def sm(out, in_):  # small copy, engine-rotated
                state["j"] += 1
                _cp(out, in_, state["j"] % 2)

            P = [dict() for _ in range(npair)]   # pair-shared tensors

            def RING():
                """[128, side(N|T), tile, 192] fp16; cols 128:192 hold the
                V (side 0) / U (side 1) columns, newest first."""
                return sb.tile([128, 2, 2, 192], f16, tag="ring", name="ring",
                               bufs=npair * 2 + 4)

            def FT(tag, bufs=npair + 2):
                return sb.tile([128, 2, 128], f16, tag=tag, name=tag,
                               bufs=bufs)

            def PG():
                """Two PSUM banks: [128, side(NV|TU), tile, 256]."""
                return ps.tile([128, 2, 2, 256], f32, tag="mg", bufs=3,
                               name="pg")

            def MM(out_ap, lhsT, rhs):
                nc.tensor.matmul(out_ap, lhsT, rhs, start=True, stop=True)

            def SMM(lhsT, rhs, w):
                p = ps.tile([128, w], f32, tag="sm", bufs=2, name="sp")
                nc.tensor.matmul(p[:], lhsT, rhs, start=True, stop=True)
                return p

            def Eap(t):
                return IN[:, t, 0:128]

            def ETap(t):
                return IN[:, t, 128:256]

            # ---- stages --------------------------------------------------
            def s_f0(t):
                # F0 = E + I (only the N form is ever used, as c-chain lhsT)
                if t % 2 == 0:
                    m = t // 2
                    F0 = FT("F0")
                    nc.gpsimd.tensor_add(F0[:], IN[:, t:t + 2, 0:128], IW[:])
                    P[m]["F0"] = F0

            def s_e2(t):
                m, h = t // 2, t % 2
                if h == 0:
                    P[m]["pE2"] = PG()
                p = P[m]["pE2"]
                MM(p[:, 0, h, 0:128], ETap(t), Eap(t))
                MM(p[:, 1, h, 0:128], Eap(t), ETap(t))
                if h == 1:
                    E2 = RING()
                    ev(E2[:, :, :, 0:128], p[:, :, :, 0:128])
                    P[m]["E2"] = E2
                    # F0sqT = E2^T + H0^T  (from psum: full fp32 E2)
                    FqT = FT("F0sqT")
                    nc.vector.tensor_add(FqT[:], p[:, 1, :, 0:128],
                                         IN[:, t - 1:t + 1, 384:512])
                    P[m]["F0sqT"] = FqT

            def s_f1(t):
                if t % 2 == 0:
                    m = t // 2
                    E2 = P[m]["E2"]
                    F1 = FT("F1")
                    nc.vector.tensor_add(F1[:], E2[:, 0, :, 0:128], IW[:])
                    P[m]["F1"] = F1

            def s_e4(t):
                m, h = t // 2, t % 2
                E2 = P[m]["E2"]
                if h == 0:
                    P[m]["pE4"] = PG()
                p = P[m]["pE4"]
                MM(p[:, 0, h, 0:128], E2[:, 1, h, 0:128], E2[:, 0, h, 0:128])
                MM(p[:, 1, h, 0:128], E2[:, 0, h, 0:128], E2[:, 1, h, 0:128])
                if h == 1:
                    E4 = RING()
                    ev(E4[:, :, :, 0:128], p[:, :, :, 0:128])
                    P[m]["E4"] = E4
                    F2 = FT("F2")
                    nc.gpsimd.tensor_add(F2[:], E4[:, 0, :, 0:128], IW[:])
                    P[m]["F2"] = F2
                    F2T = FT("F2T")
                    nc.vector.tensor_add(F2T[:], E4[:, 1, :, 0:128], IW[:])
                    P[m]["F2T"] = F2T

            def s_e8(t):
                m, h = t // 2, t % 2
                E4 = P[m]["E4"]
                if h == 0:
                    P[m]["pE8"] = PG()
                p = P[m]["pE8"]
                MM(p[:, 0, h, 0:128], E4[:, 1, h, 0:128], E4[:, 0, h, 0:128])
                if h == 1:
                    F3 = FT("F3")
                    nc.vector.tensor_add(F3[:], p[:, 0, :, 0:128], IW[:])
                    P[m]["F3"] = F3

            def s_pp(t):
                # side 0 slot: PAT = F1^T F0sq^T; side 1 slot: PB = F2 F3
                m, h = t // 2, t % 2
                d = P[m]
                if h == 0:
                    d["pPP"] = PG()
                p = d["pPP"]
                MM(p[:, 0, h, 0:128], d["F1"][:, h, :], d["F0sqT"][:, h, :])
                MM(p[:, 1, h, 0:128], d["F2T"][:, h, :], d["F3"][:, h, :])
                if h == 1:
                    PP = RING()
                    ev(PP[:, :, :, 0:128], p[:, :, :, 0:128])
                    d["PP"] = PP

            def s_nt0(t):
                m, h = t // 2, t % 2
                d = P[m]
                PP = d["PP"]
                if h == 0:
                    d["pNT0"] = PG()
                p = d["pNT0"]
                MM(p[:, 0, h, 0:128], PP[:, 0, h, 0:128], PP[:, 1, h, 0:128])
                MM(p[:, 1, h, 0:128], PP[:, 1, h, 0:128], PP[:, 0, h, 0:128])
                if h == 1:
                    R = RING()
                    ev(R[:, :, :, 0:128], p[:, :, :, 0:128])
                    # inject V0 = dtB for both tiles at V-area col 0
                    nc.gpsimd.tensor_copy(R[:, 0, 0, 128:129],
                                          BC[:, 4 * m:4 * m + 1])
                    nc.gpsimd.tensor_copy(R[:, 0, 1, 128:129],
                                          BC[:, 4 * m + 1:4 * m + 2])
                    # dummy init of side-1 col 128: level-1's padded T-MM
                    # streams it so the evacuated PSUM is fully written
                    nc.gpsimd.tensor_copy(R[:, 1, 0, 128:129],
                                          BC[:, 4 * m:4 * m + 1])
                    nc.gpsimd.tensor_copy(R[:, 1, 1, 128:129],
                                          BC[:, 4 * m + 1:4 * m + 2])
                    d["NT0"] = R

            def mk_c(ci):
                def s_c(t):
                    m, h = t // 2, t % 2
                    d = P[m]
                    if ci == 0 and h == 0:
                        d["u0"] = sb.tile([128, 2], f16, tag="u0", name="u0",
                                          bufs=npair + 1)
                    Fk = d[("F0", "F1", "F2", "F3")[ci]]
                    c = (BC[:, 4 * m + 2 + h:4 * m + 3 + h] if ci == 0
                         else d[f"c{ci - 1}"][:, h:h + 1])
                    cp = SMM(Fk[:, h, :], c, w=1)
                    if ci < 3:
                        if h == 0:
                            d[f"c{ci}"] = sb.tile([128, 2], f16, tag=f"c{ci}",
                                                  name=f"c{ci}", bufs=6)
                        nc.scalar.copy(d[f"c{ci}"][:, h:h + 1], cp[:])
                    else:
                        nc.scalar.copy(d["u0"][:, h:h + 1], cp[:])
                return s_c

            def mk_pow(k):
                # level k: NT_k = NT_{k-1}^2 (pair), with the V-apply (k<=6)
                # or U-apply (k>=7) folded into the matmuls as extra moving
                # columns.  w = number of new V/U columns this level.
                w = 1 << ((k - 1) if k <= 6 else (k - 7))
                side = 0 if k <= 6 else 1   # which ring side carries cols

                def s_pow(t):
                    m, h = t // 2, t % 2
                    d = P[m]
                    Rp = d[f"NT{k - 1}"]
                    if h == 0:
                        d[f"p{k}"] = PG()
                    p = d[f"p{k}"]
                    if k <= 6:
                        # [N_k | dA^{2^(k-1)} Vold]
                        MM(p[:, 0, h, 0:128 + w], Rp[:, 1, h, 0:128],
                           Rp[:, 0, h, 0:128 + w])
                        MM(p[:, 1, h, 0:128 + w], Rp[:, 0, h, 0:128],
                           Rp[:, 1, h, 0:128 + w])
                    elif k <= 10:
                        MM(p[:, 0, h, 0:128 + w], Rp[:, 1, h, 0:128],
                           Rp[:, 0, h, 0:128 + w])
                        # [T_k | M^{2^(k-7)T} Uold]
                        MM(p[:, 1, h, 0:128 + w], Rp[:, 0, h, 0:128],
                           Rp[:, 1, h, 0:128 + w])
                    else:  # k == 11: N-only squaring + U-apply j=4
                        MM(p[:, 0, h, 0:128], Rp[:, 1, h, 0:128],
                           Rp[:, 0, h, 0:128])
                        MM(p[:, 1, h, 0:w], Rp[:, 0, h, 0:128],
                           Rp[:, 1, h, 128:128 + w])
                    if h == 1:
                        R = RING()
                        if k <= 10:
                            ev(R[:, :, :, 0:128 + w], p[:, :, :, 0:128 + w])
                        else:
                            ev(R[:, 0, :, 0:128], p[:, 0, :, 0:128])
                            ev(R[:, 1, :, 128:128 + w], p[:, 1, :, 0:w])
                        # carry the old V/U columns (newest-first order);
                        # both sides so padded streams stay initialized
                        nc.gpsimd.tensor_copy(
                            R[:, :, :, 128 + w:128 + 2 * w],
                            Rp[:, :, :, 128:128 + w])
                        if k == 6:
                            # V complete: extract to standalone tile
                            Vf = sb.tile([128, 2, 64], f16, tag="Vf",
                                         name="Vf", bufs=npair + 1)
                            nc.gpsimd.tensor_copy(Vf[:], R[:, 0, :, 128:192])
                            d["Vfin"] = Vf
                            # inject U0 at U-area col 0 for the k=7 merge
                            nc.gpsimd.tensor_copy(R[:, 1, 0, 128:129],
                                                  d["u0"][:, 0:1])
                            nc.gpsimd.tensor_copy(R[:, 1, 1, 128:129],
                                                  d["u0"][:, 1:2])
                        d[f"NT{k}"] = R
                return s_pow

            def s_u32(t):
                # U-apply j=5: U32 = NT11^T Uold(32)
                m, h = t // 2, t % 2
                d = P[m]
                R = d["NT11"]
                if h == 0:
                    d["Ufin"] = sb.tile([128, 2, 64], f16, tag="Uf",
                                        name="Uf", bufs=npair + 1)
                up = SMM(R[:, 0, h, 0:128], R[:, 1, h, 128:160], w=32)
                sm(d["Ufin"][:, h, 0:32], up[:])
                if h == 1:
                    nc.gpsimd.tensor_copy(d["Ufin"][:, :, 32:64],
                                          R[:, 1, :, 128:160])

            def s_fin(t):
                m, h = t // 2, t % 2
                d = P[m]
                # two separate "sm" banks: the two matmuls use different PE
                # row groups and run concurrently -- same-bank PSUM writes
                # from concurrent row groups are a fatal HW collision
                pKa = ps.tile([64, 64], f32, tag="sm", bufs=2, name="pKa")
                pKb = ps.tile([64, 64], f32, tag="sm", bufs=2, name="pKb")
                nc.tensor.matmul(pKa[:], d["Ufin"][0:64, h, :],
                                 d["Vfin"][0:64, h, :], start=True, stop=True)
                nc.tensor.matmul(pKb[:], d["Ufin"][64:128, h, :],
                                 d["Vfin"][64:128, h, :], start=True,
                                 stop=True)
                sm(OUT[:, t * 128:t * 128 + 64], pKa[:])
                sm(OUT[:, t * 128 + 64:(t + 1) * 128], pKb[:])

            stages = ([s_f0, s_e2, s_f1, s_e4, s_e8] +
                      [mk_c(0), mk_c(1), mk_c(2), mk_c(3)] +
                      [s_pp, s_nt0] +
                      [mk_pow(k) for k in range(1, 12)] +
                      [s_u32, s_fin])
            ns = len(stages)
            # skewed (wavefront) emission: tile t runs stage s at step s + t.
            # Descending tile order within a step so pair-shared tensors
            # produced by the odd tile (stage s) precede the even tile's
            # stage s+1 in the same step.
            for step in range(ns + nt - 1):
                for t in reversed(range(nt)):
                    s = step - t
                    if 0 <= s < ns:
                        stages[s](t)

            # output DMAs (2 chunks)
            half = nt // 2 * 128
            nc.scalar.dma_start(Y[:, 0:half], OUT[:, 0:half])
            nc.sync.dma_start(Y[:, half:2 * half], OUT[:, half:2 * half])
            rep.__exit__(None, None, None)

    nc.compile()
    return nc


def _host_pack(A, B, C, log_dt):
    A = np.asarray(A, np.float32)
    B = np.asarray(B, np.float32)
    C = np.asarray(C, np.float32)
    log_dt = np.asarray(log_dt, np.float32)
    dt = np.exp(log_dt)
    E = (0.5 * dt)[:, None, None].astype(np.float32) * A      # [H,64,64]
    ETc = np.swapaxes(E, 1, 2)
    dtB = (dt[:, None] * B).astype(np.float32)

    G = A.shape[0] // 2   # number of block-diag tiles, all cores
    I64 = np.eye(64, dtype=np.float32)

    def blockdiag(M):  # [H,64,64] -> [G,128,128] fp16
        out = np.zeros((G, 128, 128), np.float16)
        out[:, 0:64, 0:64] = M[0::2]
        out[:, 64:128, 64:128] = M[1::2]
        return out

    Epk = blockdiag(E)
    ETpk = blockdiag(ETc)
    H0pk = blockdiag(2.0 * E + I64)
    H0Tpk = blockdiag(2.0 * ETc + I64)
    # IN layout per tile: [E | ET | H0 | H0T] -> [G, 128, 512]
    INpk = np.concatenate([Epk, ETpk, H0pk, H0Tpk], axis=2)
    BCt = np.zeros((G, 128, 2), np.float16)   # per tile [dtB | C]
    BCt[:, 0:64, 0] = dtB[0::2]
    BCt[:, 64:128, 0] = dtB[1::2]
    BCt[:, 0:64, 1] = C[0::2]
    BCt[:, 64:128, 1] = C[1::2]
    # per pair: [dtB_a | dtB_b | C_a | C_b]
    BCpk = np.zeros((G // 2, 128, 4), np.float16)
    BCpk[:, :, 0] = BCt[0::2, :, 0]
    BCpk[:, :, 1] = BCt[1::2, :, 0]
    BCpk[:, :, 2] = BCt[0::2, :, 1]
    BCpk[:, :, 3] = BCt[1::2, :, 1]
    I128 = np.eye(128, dtype=np.float16)
    IWpk = np.concatenate([I128, I128], axis=1)   # [128, 256]
    return INpk, BCpk, IWpk


def _in_maps(A, B, C, log_dt):
    INpk, BCpk, IWpk = _host_pack(A, B, C, log_dt)
    maps = []
    for c in range(NCORES):
        s = slice(c * NT, (c + 1) * NT)
        INc = np.ascontiguousarray(
            INpk[s].transpose(1, 0, 2).reshape(128, NT * 512))
        sp = slice(c * NP, (c + 1) * NP)
        BCc = np.ascontiguousarray(
            BCpk[sp].transpose(1, 0, 2).reshape(128, NT * 2))
        maps.append({"INd": INc, "BCd": BCc, "IWd": IWpk})
    return maps


def kernel(A, B, C, log_dt, L):
    from concourse.bass_utils import run_bass_kernel_spmd

    if "nc" not in _cache:
        _cache["nc"] = _build_program(NT)
    nc = _cache["nc"]

    res = run_bass_kernel_spmd(nc, _in_maps(A, B, C, log_dt),
                               core_ids=list(range(NCORES)))
    K = np.empty((H, L), np.float32)
    for c in range(NCORES):
        Yc = res.results[c]["Y"]            # [64, NT*128]
        for t in range(NT):
            blk = Yc[:, t * 128:(t + 1) * 128]
            # U/V columns are stored newest-first => both axes reversed
            K[c * CH_PER_CORE + 2 * t] = blk[::-1, 0:64][:, ::-1].reshape(L)
            K[c * CH_PER_CORE + 2 * t + 1] = (
                blk[::-1, 64:128][:, ::-1].reshape(L))
    return K


# revision 16
# speedup vs baseline: 3.9976x; 2.0624x over previous
"""Trainium2 Bass kernel for the Krylov/SSM problem.

K[h, l] = C[h] @ dA[h]^l @ dB[h],  l = 0..L-1
  dA = (I - (dt/2)A)^-1 (I + (dt/2)A),  dB = dt * (I - (dt/2)A)^-1 B

Device algorithm (per channel):
  E = (dt/2) A           (host prep, packed 2 channels per 128x128 block-diag tile)
  Neumann-product inverse: G = (I+E)(I+E^2)(I+E^4)(I+E^8)   [rho(E) <= ~0.25]
  dA  = (I+E)^2 (I+E^2)(I+E^4)(I+E^8)
  Ct  = G^T C            (solve folded into the C side; G commutes with dA)
  V   = [dt*B, dA(dt*B), ..., dA^63(dt*B)]        (doubling)
  U   = [Ct, M^T Ct, ..., (M^T)^63 Ct], M = dA^64 (doubling, powers to dA^2048)
  K[h, 64j + i] = (U^T V)[j, i]

All data is fp16 (PSUM accumulation is fp32); numpy simulation of the full
fp16 pipeline gives rel err ~8e-3 vs the fp32 reference (tolerance 2e-2).
fp16 matmuls stream at 1 cycle/row on the PE (4x faster than fp32) and
128-col fp16 stationaries get fast-weight-load.

The matmul primitive computes lhsT.T @ rhs, so the squaring chain keeps both
X and X^T for every power ("pair squaring").  The V/U doubling applies are
FOLDED into the chain matmuls: the apply at level k uses the same stationary
operand as the squaring (T_{k-1} for V / N_{k-1} for U), so the moving
operand is just widened to [N_{k-1} | V-cols] / [T_{k-1} | U-cols].  V/U
columns therefore live INSIDE the ring tiles, appended newest-first (the
final column order is fully reversed; the host un-reverses when unpacking).

Ring layout [128, 2(side N/T), 2(tile), 192]: per tile, side 0 = [N | Vcols],
side 1 = [T | Ucols].  Each level's products land in one two-bank PSUM tile
([128, 2(side), 2(tile), 256]) evacuated by a single 4D strided copy; the
old V/U columns are carried forward by a small Pool-engine copy.

Tiles are processed in PAIRS (the two tiles of a pair share PSUM tiles and
wide evacuations).  Instructions are emitted in a skewed wavefront across
the 16 per-core tiles, descending tile order within a step so pair-shared
tensors produced by the odd tile precede the even tile's next stage.

Sharding: H axis, 32 channels (16 tiles) per core across 8 cores. SPMD, no
communication.
"""

import numpy as np

H, N, L = 256, 64, 4096
NCORES = 8
CH_PER_CORE = H // NCORES   # 32
NT = CH_PER_CORE // 2       # 16 block-diag tiles per core
NP = NT // 2                # 8 tile pairs

_cache = {}


def _build_program(nt=NT, repeat=None):
    import contextlib
    import concourse.bacc as bacc
    import concourse.tile as tile
    import concourse.mybir as mybir

    f16 = mybir.dt.float16
    f32 = mybir.dt.float32
    npair = nt // 2
    nc = bacc.Bacc("TRN2", target_bir_lowering=False, debug=False)

    # IN cols per tile: [E | ET | H0 | H0T], fp16
    INd = nc.dram_tensor("INd", [128, nt * 512], f16, kind="ExternalInput").ap()
    # BC cols per pair: [dtB_a | dtB_b | C_a | C_b]
    BCd = nc.dram_tensor("BCd", [128, nt * 2], f16, kind="ExternalInput").ap()
    IWd = nc.dram_tensor("IWd", [128, 256], f16, kind="ExternalInput").ap()
    Y = nc.dram_tensor("Y", [64, nt * 128], f32, kind="ExternalOutput").ap()

    with tile.TileContext(nc) as tc:
        with (
            tc.tile_pool(name="const", bufs=1) as cpool,
            tc.tile_pool(name="sb", bufs=1) as sb,
            tc.tile_pool(name="ps", bufs=1, space="PSUM") as ps,
        ):
            IN = cpool.tile([128, nt, 512], f16, name="IN")
            BC = cpool.tile([128, nt * 2], f16, name="BC")
            IW = cpool.tile([128, 2, 128], f16, name="IW")
            OUT = cpool.tile([64, nt * 128], f32, name="OUT")
            nc.sync.dma_start(IW[:], IWd)
            nc.sync.dma_start(BC[:], BCd)

            rep = tc.For_i(0, repeat, 1) if repeat else contextlib.nullcontext()
            rep.__enter__()

            # input DMAs: 4 chunks of 4 tiles on the 3 DMA-capable queues
            chunk = nt // 4
            for ci, eng in enumerate((nc.sync, nc.gpsimd, nc.scalar,
                                      nc.gpsimd)):
                eng.dma_start(IN[:, ci * chunk:(ci + 1) * chunk, :],
                              INd[:, ci * chunk * 512:(ci + 1) * chunk * 512])

            state = {"i": 0, "j": 0}

            def _cp(out, in_, dve):
                if dve:
                    nc.vector.tensor_copy(out, in_)
                else:
                    nc.scalar.copy(out, in_)

            def ev(out, in_):  # evac copy, engine-rotated (DVE : ACT)
                state["i"] += 1
                _cp(out, in_, state["i"] % 2)

            def sm(out, in_):  # small copy, engine-rotated
                state["j"] += 1
                _cp(out, in_, state["j"] % 2)

            P = [dict() for _ in range(npair)]   # pair-shared tensors

            def RING():
                """[128, side(N|T), tile, 192] fp16; cols 128:192 hold the
                V (side 0) / U (side 1) columns, newest first."""
                return sb.tile([128, 2, 2, 192], f16, tag="ring", name="ring",
                               bufs=npair * 2 + 4)

            def FT(tag, bufs=npair + 2):
                return sb.tile([128, 2, 128], f16, tag=tag, name=tag,
                               bufs=bufs)

            def PG():
                """Two PSUM banks: [128, side(NV|TU), tile, 256]."""
                return ps.tile([128, 2, 2, 256], f32, tag="mg", bufs=3,
                               name="pg")

            def MM(out_ap, lhsT, rhs):
                nc.tensor.matmul(out_ap, lhsT, rhs, start=True, stop=True)

            def SMM(lhsT, rhs, w):
                p = ps.tile([128, w], f32, tag="sm", bufs=2, name="sp")
                nc.tensor.matmul(p[:], lhsT, rhs, start=True, stop=True)
                return p

            def Eap(t):
                return IN[:, t, 0:128]

            def ETap(t):
                return IN[:, t, 128:256]

            # ---- stages --------------------------------------------------
            def s_f0(t):
                # F0 = E + I (only the N form is ever used, as c-chain lhsT)
                if t % 2 == 0:
                    m = t // 2
                    F0 = FT("F0")
                    nc.gpsimd.tensor_add(F0[:], IN[:, t:t + 2, 0:128], IW[:])
                    P[m]["F0"] = F0

            def s_e2(t):
                m, h = t // 2, t % 2
                if h == 0:
                    P[m]["pE2"] = PG()
                p = P[m]["pE2"]
                MM(p[:, 0, h, 0:128], ETap(t), Eap(t))
                MM(p[:, 1, h, 0:128], Eap(t), ETap(t))
                if h == 1:
                    E2 = RING()
                    ev(E2[:, :, :, 0:128], p[:, :, :, 0:128])
                    P[m]["E2"] = E2
                    # F0sqT = E2^T + H0^T  (ring-sourced: all-fp16 SBUF op
                    # runs in DVE 2x mode and releases the PSUM bank sooner)
                    FqT = FT("F0sqT")
                    nc.vector.tensor_add(FqT[:], E2[:, 1, :, 0:128],
                                         IN[:, t - 1:t + 1, 384:512])
                    P[m]["F0sqT"] = FqT

            def s_f1(t):
                if t % 2 == 0:
                    m = t // 2
                    E2 = P[m]["E2"]
                    F1 = FT("F1")
                    nc.vector.tensor_add(F1[:], E2[:, 0, :, 0:128], IW[:])
                    P[m]["F1"] = F1

            def s_e4(t):
                m, h = t // 2, t % 2
                E2 = P[m]["E2"]
                if h == 0:
                    P[m]["pE4"] = PG()
                p = P[m]["pE4"]
                MM(p[:, 0, h, 0:128], E2[:, 1, h, 0:128], E2[:, 0, h, 0:128])
                MM(p[:, 1, h, 0:128], E2[:, 0, h, 0:128], E2[:, 1, h, 0:128])
                if h == 1:
                    E4 = RING()
                    ev(E4[:, :, :, 0:128], p[:, :, :, 0:128])
                    P[m]["E4"] = E4
                    F2 = FT("F2")
                    nc.gpsimd.tensor_add(F2[:], E4[:, 0, :, 0:128], IW[:])
                    P[m]["F2"] = F2
                    F2T = FT("F2T")
                    nc.vector.tensor_add(F2T[:], E4[:, 1, :, 0:128], IW[:])
                    P[m]["F2T"] = F2T

            def s_e8(t):
                m, h = t // 2, t % 2
                E4 = P[m]["E4"]
                if h == 0:
                    P[m]["pE8"] = PG()
                p = P[m]["pE8"]
                MM(p[:, 0, h, 0:128], E4[:, 1, h, 0:128], E4[:, 0, h, 0:128])
                if h == 1:
                    F3 = FT("F3")
                    nc.vector.tensor_add(F3[:], p[:, 0, :, 0:128], IW[:])
                    P[m]["F3"] = F3

            def s_pp(t):
                # side 0 slot: PAT = F1^T F0sq^T; side 1 slot: PB = F2 F3
                m, h = t // 2, t % 2
                d = P[m]
                if h == 0:
                    d["pPP"] = PG()
                p = d["pPP"]
                MM(p[:, 0, h, 0:128], d["F1"][:, h, :], d["F0sqT"][:, h, :])
                MM(p[:, 1, h, 0:128], d["F2T"][:, h, :], d["F3"][:, h, :])
                if h == 1:
                    PP = RING()
                    ev(PP[:, :, :, 0:128], p[:, :, :, 0:128])
                    d["PP"] = PP

            def s_nt0(t):
                m, h = t // 2, t % 2
                d = P[m]
                PP = d["PP"]
                if h == 0:
                    d["pNT0"] = PG()
                p = d["pNT0"]
                MM(p[:, 0, h, 0:128], PP[:, 0, h, 0:128], PP[:, 1, h, 0:128])
                MM(p[:, 1, h, 0:128], PP[:, 1, h, 0:128], PP[:, 0, h, 0:128])
                if h == 1:
                    R = RING()
                    ev(R[:, :, :, 0:128], p[:, :, :, 0:128])
                    # inject V0 = dtB for both tiles at V-area col 0
                    nc.gpsimd.tensor_copy(R[:, 0, 0, 128:129],
                                          BC[:, 4 * m:4 * m + 1])
                    nc.gpsimd.tensor_copy(R[:, 0, 1, 128:129],
                                          BC[:, 4 * m + 1:4 * m + 2])
                    # dummy init of side-1 col 128: level-1's padded T-MM
                    # streams it so the evacuated PSUM is fully written
                    nc.gpsimd.tensor_copy(R[:, 1, 0, 128:129],
                                          BC[:, 4 * m:4 * m + 1])
                    nc.gpsimd.tensor_copy(R[:, 1, 1, 128:129],
                                          BC[:, 4 * m + 1:4 * m + 2])
                    d["NT0"] = R

            def mk_c(ci):
                def s_c(t):
                    m, h = t // 2, t % 2
                    d = P[m]
                    if ci == 0 and h == 0:
                        d["u0"] = sb.tile([128, 2], f16, tag="u0", name="u0",
                                          bufs=npair + 1)
                    Fk = d[("F0", "F1", "F2", "F3")[ci]]
                    c = (BC[:, 4 * m + 2 + h:4 * m + 3 + h] if ci == 0
                         else d[f"c{ci - 1}"][:, h:h + 1])
                    cp = SMM(Fk[:, h, :], c, w=1)
                    if ci < 3:
                        if h == 0:
                            d[f"c{ci}"] = sb.tile([128, 2], f16, tag=f"c{ci}",
                                                  name=f"c{ci}", bufs=6)
                        nc.scalar.copy(d[f"c{ci}"][:, h:h + 1], cp[:])
                    else:
                        nc.scalar.copy(d["u0"][:, h:h + 1], cp[:])
                return s_c

            def mk_pow(k):
                # level k: NT_k = NT_{k-1}^2 (pair), with the V-apply (k<=6)
                # or U-apply (k>=7) folded into the matmuls as extra moving
                # columns.  w = number of new V/U columns this level.
                w = 1 << ((k - 1) if k <= 6 else (k - 7))
                side = 0 if k <= 6 else 1   # which ring side carries cols

                def s_pow(t):
                    m, h = t // 2, t % 2
                    d = P[m]
                    Rp = d[f"NT{k - 1}"]
                    if h == 0:
                        d[f"p{k}"] = PG()
                    p = d[f"p{k}"]
                    if k <= 6:
                        # [N_k | dA^{2^(k-1)} Vold]
                        MM(p[:, 0, h, 0:128 + w], Rp[:, 1, h, 0:128],
                           Rp[:, 0, h, 0:128 + w])
                        MM(p[:, 1, h, 0:128 + w], Rp[:, 0, h, 0:128],
                           Rp[:, 1, h, 0:128 + w])
                    elif k <= 10:
                        MM(p[:, 0, h, 0:128 + w], Rp[:, 1, h, 0:128],
                           Rp[:, 0, h, 0:128 + w])
                        # [T_k | M^{2^(k-7)T} Uold]
                        MM(p[:, 1, h, 0:128 + w], Rp[:, 0, h, 0:128],
                           Rp[:, 1, h, 0:128 + w])
                    else:  # k == 11: N-only squaring + U-apply j=4
                        MM(p[:, 0, h, 0:128], Rp[:, 1, h, 0:128],
                           Rp[:, 0, h, 0:128])
                        MM(p[:, 1, h, 0:w], Rp[:, 0, h, 0:128],
                           Rp[:, 1, h, 128:128 + w])
                    if h == 1:
                        R = RING()
                        if k <= 10:
                            ev(R[:, :, :, 0:128 + w], p[:, :, :, 0:128 + w])
                        else:
                            ev(R[:, 0, :, 0:128], p[:, 0, :, 0:128])
                            ev(R[:, 1, :, 128:128 + w], p[:, 1, :, 0:w])
                        # carry the old V/U columns (newest-first order);
                        # both sides so padded streams stay initialized
                        nc.gpsimd.tensor_copy(
                            R[:, :, :, 128 + w:128 + 2 * w],
                            Rp[:, :, :, 128:128 + w])
                        if k == 6:
                            # V complete: extract to standalone tile
                            Vf = sb.tile([128, 2, 64], f16, tag="Vf",
                                         name="Vf", bufs=npair + 1)
                            nc.gpsimd.tensor_copy(Vf[:], R[:, 0, :, 128:192])
                            d["Vfin"] = Vf
                            # inject U0 at U-area col 0 for the k=7 merge
                            nc.gpsimd.tensor_copy(R[:, 1, 0, 128:129],
                                                  d["u0"][:, 0:1])
                            nc.gpsimd.tensor_copy(R[:, 1, 1, 128:129],
                                                  d["u0"][:, 1:2])
                        d[f"NT{k}"] = R
                return s_pow

            def s_u32(t):
                # U-apply j=5: U32 = NT11^T Uold(32)
                m, h = t // 2, t % 2
                d = P[m]
                R = d["NT11"]
                if h == 0:
                    d["Ufin"] = sb.tile([128, 2, 64], f16, tag="Uf",
                                        name="Uf", bufs=npair + 1)
                up = SMM(R[:, 0, h, 0:128], R[:, 1, h, 128:160], w=32)
                sm(d["Ufin"][:, h, 0:32], up[:])
                if h == 1:
                    nc.gpsimd.tensor_copy(d["Ufin"][:, :, 32:64],
                                          R[:, 1, :, 128:160])

            def s_fin(t):
                m, h = t // 2, t % 2
                d = P[m]
                # two separate "sm" banks: the two matmuls use different PE
                # row groups and run concurrently -- same-bank PSUM writes
                # from concurrent row groups are a fatal HW collision
                pKa = ps.tile([64, 64], f32, tag="sm", bufs=2, name="pKa")
                pKb = ps.tile([64, 64], f32, tag="sm", bufs=2, name="pKb")
                nc.tensor.matmul(pKa[:], d["Ufin"][0:64, h, :],
                                 d["Vfin"][0:64, h, :], start=True, stop=True)
                nc.tensor.matmul(pKb[:], d["Ufin"][64:128, h, :],
                                 d["Vfin"][64:128, h, :], start=True,
                                 stop=True)
                sm(OUT[:, t * 128:t * 128 + 64], pKa[:])
                sm(OUT[:, t * 128 + 64:(t + 1) * 128], pKb[:])

            stages = ([s_f0, s_e2, s_f1, s_e4, s_e8] +
                      [mk_c(0), mk_c(1), mk_c(2), mk_c(3)] +
                      [s_pp, s_nt0] +
                      [mk_pow(k) for k in range(1, 12)] +
                      [s_u32, s_fin])
            ns = len(stages)
            # skewed (wavefront) emission: tile t runs stage s at step s + t.
            # Descending tile order within a step so pair-shared tensors
            # produced by the odd tile (stage s) precede the even tile's
            # stage s+1 in the same step.
            for step in range(ns + nt - 1):
                for t in reversed(range(nt)):
                    s = step - t
                    if 0 <= s < ns:
                        stages[s](t)

            # output DMAs (2 chunks)
            half = nt // 2 * 128
            nc.scalar.dma_start(Y[:, 0:half], OUT[:, 0:half])
            nc.sync.dma_start(Y[:, half:2 * half], OUT[:, half:2 * half])
            rep.__exit__(None, None, None)

    nc.compile()
    return nc


def _host_pack(A, B, C, log_dt):
    A = np.asarray(A, np.float32)
    B = np.asarray(B, np.float32)
    C = np.asarray(C, np.float32)
    log_dt = np.asarray(log_dt, np.float32)
    dt = np.exp(log_dt)
    E = (0.5 * dt)[:, None, None].astype(np.float32) * A      # [H,64,64]
    ETc = np.swapaxes(E, 1, 2)
    dtB = (dt[:, None] * B).astype(np.float32)

    G = A.shape[0] // 2   # number of block-diag tiles, all cores
    I64 = np.eye(64, dtype=np.float32)

    def blockdiag(M):  # [H,64,64] -> [G,128,128] fp16
        out = np.zeros((G, 128, 128), np.float16)
        out[:, 0:64, 0:64] = M[0::2]
        out[:, 64:128, 64:128] = M[1::2]
        return out

    Epk = blockdiag(E)
    ETpk = blockdiag(ETc)
    H0pk = blockdiag(2.0 * E + I64)
    H0Tpk = blockdiag(2.0 * ETc + I64)
    # IN layout per tile: [E | ET | H0 | H0T] -> [G, 128, 512]
    INpk = np.concatenate([Epk, ETpk, H0pk, H0Tpk], axis=2)
    BCt = np.zeros((G, 128, 2), np.float16)   # per tile [dtB | C]
    BCt[:, 0:64, 0] = dtB[0::2]
    BCt[:, 64:128, 0] = dtB[1::2]
    BCt[:, 0:64, 1] = C[0::2]
    BCt[:, 64:128, 1] = C[1::2]
    # per pair: [dtB_a | dtB_b | C_a | C_b]
    BCpk = np.zeros((G // 2, 128, 4), np.float16)
    BCpk[:, :, 0] = BCt[0::2, :, 0]
    BCpk[:, :, 1] = BCt[1::2, :, 0]
    BCpk[:, :, 2] = BCt[0::2, :, 1]
    BCpk[:, :, 3] = BCt[1::2, :, 1]
    I128 = np.eye(128, dtype=np.float16)
    IWpk = np.concatenate([I128, I128], axis=1)   # [128, 256]
    return INpk, BCpk, IWpk


def _in_maps(A, B, C, log_dt):
    INpk, BCpk, IWpk = _host_pack(A, B, C, log_dt)
    maps = []
    for c in range(NCORES):
        s = slice(c * NT, (c + 1) * NT)
        INc = np.ascontiguousarray(
            INpk[s].transpose(1, 0, 2).reshape(128, NT * 512))
        sp = slice(c * NP, (c + 1) * NP)
        BCc = np.ascontiguousarray(
            BCpk[sp].transpose(1, 0, 2).reshape(128, NT * 2))
        maps.append({"INd": INc, "BCd": BCc, "IWd": IWpk})
    return maps


def kernel(A, B, C, log_dt, L):
    from concourse.bass_utils import run_bass_kernel_spmd

    if "nc" not in _cache:
        _cache["nc"] = _build_program(NT)
    nc = _cache["nc"]

    res = run_bass_kernel_spmd(nc, _in_maps(A, B, C, log_dt),
                               core_ids=list(range(NCORES)))
    K = np.empty((H, L), np.float32)
    for c in range(NCORES):
        Yc = res.results[c]["Y"]            # [64, NT*128]
        for t in range(NT):
            blk = Yc[:, t * 128:(t + 1) * 128]
            # U/V columns are stored newest-first => both axes reversed
            K[c * CH_PER_CORE + 2 * t] = blk[::-1, 0:64][:, ::-1].reshape(L)
            K[c * CH_PER_CORE + 2 * t + 1] = (
                blk[::-1, 64:128][:, ::-1].reshape(L))
    return K


# revision 17
# speedup vs baseline: 10.5935x; 2.6500x over previous
"""Trainium2 Bass kernel for the Krylov/SSM problem.

K[h, l] = C[h] @ dA[h]^l @ dB[h],  l = 0..L-1
  dA = (I - (dt/2)A)^-1 (I + (dt/2)A),  dB = dt * (I - (dt/2)A)^-1 B

Device algorithm (per channel):
  E = (dt/2) A           (host prep, packed 2 channels per 128x128 block-diag tile)
  Neumann-product inverse: G = (I+E)(I+E^2)(I+E^4)(I+E^8)   [rho(E) <= ~0.25]
  dA  = (I+E)^2 (I+E^2)(I+E^4)(I+E^8)
  Ct  = G^T C            (solve folded into the C side; G commutes with dA)
  V   = [dt*B, dA(dt*B), ..., dA^63(dt*B)]        (doubling)
  U   = [Ct, M^T Ct, ..., (M^T)^63 Ct], M = dA^64 (doubling, powers to dA^2048)
  K[h, 64j + i] = (U^T V)[j, i]

All data is fp16 (PSUM accumulation is fp32); numpy simulation of the full
fp16 pipeline gives rel err ~8e-3 vs the fp32 reference (tolerance 2e-2).
fp16 matmuls stream at 1 cycle/row on the PE (4x faster than fp32) and
128-col fp16 stationaries get fast-weight-load.

The matmul primitive computes lhsT.T @ rhs, so the squaring chain keeps both
X and X^T for every power ("pair squaring").  The V/U doubling applies are
FOLDED into the chain matmuls: the apply at level k uses the same stationary
operand as the squaring (T_{k-1} for V / N_{k-1} for U), so the moving
operand is just widened to [N_{k-1} | V-cols] / [T_{k-1} | U-cols].  V/U
columns therefore live INSIDE the ring tiles, appended newest-first (the
final column order is fully reversed; the host un-reverses when unpacking).

Ring layout [128, 2(side N/T), 2(tile), 192]: per tile, side 0 = [N | Vcols],
side 1 = [T | Ucols].  Each level's products land in one two-bank PSUM tile
([128, 2(side), 2(tile), 256]) evacuated by a single 4D strided copy; the
old V/U columns are carried forward by a small Pool-engine copy.

Tiles are processed in PAIRS (the two tiles of a pair share PSUM tiles and
wide evacuations).  Instructions are emitted in a skewed wavefront across
the 16 per-core tiles, descending tile order within a step so pair-shared
tensors produced by the odd tile precede the even tile's next stage.

Sharding: H axis, 32 channels (16 tiles) per core across 8 cores. SPMD, no
communication.
"""

import numpy as np

H, N, L = 256, 64, 4096
NCORES = 8
CH_PER_CORE = H // NCORES   # 32
NT = CH_PER_CORE // 2       # 16 block-diag tiles per core
NP = NT // 2                # 8 tile pairs

_cache = {}


def _build_program(nt=NT, repeat=None):
    import contextlib
    import concourse.bacc as bacc
    import concourse.tile as tile
    import concourse.mybir as mybir

    f16 = mybir.dt.float16
    f32 = mybir.dt.float32
    npair = nt // 2
    nc = bacc.Bacc("TRN2", target_bir_lowering=False, debug=False)

    # IN cols per tile: [E | ET | H0 | H0T], fp16
    INd = nc.dram_tensor("INd", [128, nt * 512], f16, kind="ExternalInput").ap()
    # BC cols per pair: [dtB_a | dtB_b | C_a | C_b]
    BCd = nc.dram_tensor("BCd", [128, nt * 2], f16, kind="ExternalInput").ap()
    IWd = nc.dram_tensor("IWd", [128, 256], f16, kind="ExternalInput").ap()
    Y = nc.dram_tensor("Y", [64, nt * 128], f32, kind="ExternalOutput").ap()

    with tile.TileContext(nc) as tc:
        with (
            tc.tile_pool(name="const", bufs=1) as cpool,
            tc.tile_pool(name="sb", bufs=1) as sb,
            tc.tile_pool(name="ps", bufs=1, space="PSUM") as ps,
        ):
            IN = cpool.tile([128, nt, 512], f16, name="IN")
            BC = cpool.tile([128, nt * 2], f16, name="BC")
            IW = cpool.tile([128, 2, 128], f16, name="IW")
            OUT = cpool.tile([64, nt * 128], f32, name="OUT")
            nc.sync.dma_start(IW[:], IWd)
            nc.sync.dma_start(BC[:], BCd)

            rep = tc.For_i(0, repeat, 1) if repeat else contextlib.nullcontext()
            rep.__enter__()

            # input DMAs: 4 chunks of 4 tiles on the 3 DMA-capable queues
            chunk = nt // 4
            for ci, eng in enumerate((nc.sync, nc.gpsimd, nc.scalar,
                                      nc.gpsimd)):
                eng.dma_start(IN[:, ci * chunk:(ci + 1) * chunk, :],
                              INd[:, ci * chunk * 512:(ci + 1) * chunk * 512])

            state = {"i": 0, "j": 0}

            def _cp(out, in_, dve):
                if dve:
                    nc.vector.tensor_copy(out, in_)
                else:
                    nc.scalar.copy(out, in_)

            def ev(out, in_):  # evac copy, engine-rotated (DVE : ACT)
                state["i"] += 1
                _cp(out, in_, state["i"] % 2)

            def sm(out, in_):  # small copy, engine-rotated
                state["j"] += 1
                _cp(out, in_, state["j"] % 2)

            P = [dict() for _ in range(npair)]   # pair-shared tensors

            def RING():
                """[128, side(N|T), tile, 192] fp16; cols 128:192 hold the
                V (side 0) / U (side 1) columns, newest first."""
                return sb.tile([128, 2, 2, 192], f16, tag="ring", name="ring",
                               bufs=npair * 2 + 4)

            def FT(tag, bufs=npair + 2):
                return sb.tile([128, 2, 128], f16, tag=tag, name=tag,
                               bufs=bufs)

            def PG():
                """Two PSUM banks: [128, side(NV|TU), tile, 256]."""
                return ps.tile([128, 2, 2, 256], f32, tag="mg", bufs=3,
                               name="pg")

            def MM(out_ap, lhsT, rhs):
                nc.tensor.matmul(out_ap, lhsT, rhs, start=True, stop=True)

            def SMM(lhsT, rhs, w):
                p = ps.tile([128, w], f32, tag="sm", bufs=2, name="sp")
                nc.tensor.matmul(p[:], lhsT, rhs, start=True, stop=True)
                return p

            def Eap(t):
                return IN[:, t, 0:128]

            def ETap(t):
                return IN[:, t, 128:256]

            # ---- stages --------------------------------------------------
            def s_f0(t):
                # F0 = E + I (only the N form is ever used, as c-chain lhsT)
                if t % 2 == 0:
                    m = t // 2
                    F0 = FT("F0")
                    nc.gpsimd.tensor_add(F0[:], IN[:, t:t + 2, 0:128], IW[:])
                    P[m]["F0"] = F0

            def s_e2(t):
                m, h = t // 2, t % 2
                if h == 0:
                    P[m]["pE2"] = PG()
                p = P[m]["pE2"]
                MM(p[:, 0, h, 0:128], ETap(t), Eap(t))
                MM(p[:, 1, h, 0:128], Eap(t), ETap(t))
                if h == 1:
                    E2 = RING()
                    ev(E2[:, :, :, 0:128], p[:, :, :, 0:128])
                    P[m]["E2"] = E2
                    # F0sqT = E2^T + H0^T  (ring-sourced: all-fp16 SBUF op
                    # runs in DVE 2x mode and releases the PSUM bank sooner)
                    FqT = FT("F0sqT")
                    nc.vector.tensor_add(FqT[:], E2[:, 1, :, 0:128],
                                         IN[:, t - 1:t + 1, 384:512])
                    P[m]["F0sqT"] = FqT

            def s_f1(t):
                if t % 2 == 0:
                    m = t // 2
                    E2 = P[m]["E2"]
                    F1 = FT("F1")
                    nc.vector.tensor_add(F1[:], E2[:, 0, :, 0:128], IW[:])
                    P[m]["F1"] = F1

            def s_e4(t):
                m, h = t // 2, t % 2
                E2 = P[m]["E2"]
                if h == 0:
                    P[m]["pE4"] = PG()
                p = P[m]["pE4"]
                MM(p[:, 0, h, 0:128], E2[:, 1, h, 0:128], E2[:, 0, h, 0:128])
                MM(p[:, 1, h, 0:128], E2[:, 0, h, 0:128], E2[:, 1, h, 0:128])
                if h == 1:
                    E4 = RING()
                    ev(E4[:, :, :, 0:128], p[:, :, :, 0:128])
                    P[m]["E4"] = E4
                    F2 = FT("F2")
                    nc.gpsimd.tensor_add(F2[:], E4[:, 0, :, 0:128], IW[:])
                    P[m]["F2"] = F2
                    F2T = FT("F2T")
                    nc.vector.tensor_add(F2T[:], E4[:, 1, :, 0:128], IW[:])
                    P[m]["F2T"] = F2T

            def s_e8(t):
                # F3 = E4^2 + I, the +I accumulated on the PE (ident @ ident)
                # so the PSUM consumer is a plain evacuation copy
                m, h = t // 2, t % 2
                E4 = P[m]["E4"]
                if h == 0:
                    P[m]["pE8"] = PG()
                p = P[m]["pE8"]
                nc.tensor.matmul(p[:, 0, h, 0:128], E4[:, 1, h, 0:128],
                                 E4[:, 0, h, 0:128], start=True, stop=False)
                nc.tensor.matmul(p[:, 0, h, 0:128], IW[:, 0, :], IW[:, 0, :],
                                 start=False, stop=True)
                if h == 1:
                    F3 = FT("F3")
                    ev(F3[:], p[:, 0, :, 0:128])
                    P[m]["F3"] = F3

            def s_pp(t):
                # side 0 slot: PAT = F1^T F0sq^T; side 1 slot: PB = F2 F3
                m, h = t // 2, t % 2
                d = P[m]
                if h == 0:
                    d["pPP"] = PG()
                p = d["pPP"]
                MM(p[:, 0, h, 0:128], d["F1"][:, h, :], d["F0sqT"][:, h, :])
                MM(p[:, 1, h, 0:128], d["F2T"][:, h, :], d["F3"][:, h, :])
                if h == 1:
                    PP = RING()
                    ev(PP[:, :, :, 0:128], p[:, :, :, 0:128])
                    d["PP"] = PP

            def s_nt0(t):
                m, h = t // 2, t % 2
                d = P[m]
                PP = d["PP"]
                if h == 0:
                    d["pNT0"] = PG()
                p = d["pNT0"]
                MM(p[:, 0, h, 0:128], PP[:, 0, h, 0:128], PP[:, 1, h, 0:128])
                MM(p[:, 1, h, 0:128], PP[:, 1, h, 0:128], PP[:, 0, h, 0:128])
                if h == 1:
                    R = RING()
                    ev(R[:, :, :, 0:128], p[:, :, :, 0:128])
                    # inject V0 = dtB for both tiles at V-area col 0
                    nc.gpsimd.tensor_copy(R[:, 0, 0, 128:129],
                                          BC[:, 4 * m:4 * m + 1])
                    nc.gpsimd.tensor_copy(R[:, 0, 1, 128:129],
                                          BC[:, 4 * m + 1:4 * m + 2])
                    # dummy init of side-1 col 128: level-1's padded T-MM
                    # streams it so the evacuated PSUM is fully written
                    nc.gpsimd.tensor_copy(R[:, 1, 0, 128:129],
                                          BC[:, 4 * m:4 * m + 1])
                    nc.gpsimd.tensor_copy(R[:, 1, 1, 128:129],
                                          BC[:, 4 * m + 1:4 * m + 2])
                    d["NT0"] = R

            def mk_c(ci):
                def s_c(t):
                    m, h = t // 2, t % 2
                    d = P[m]
                    if ci == 0 and h == 0:
                        d["u0"] = sb.tile([128, 2], f16, tag="u0", name="u0",
                                          bufs=npair + 1)
                    Fk = d[("F0", "F1", "F2", "F3")[ci]]
                    c = (BC[:, 4 * m + 2 + h:4 * m + 3 + h] if ci == 0
                         else d[f"c{ci - 1}"][:, h:h + 1])
                    cp = SMM(Fk[:, h, :], c, w=1)
                    if ci < 3:
                        if h == 0:
                            d[f"c{ci}"] = sb.tile([128, 2], f16, tag=f"c{ci}",
                                                  name=f"c{ci}", bufs=6)
                        nc.scalar.copy(d[f"c{ci}"][:, h:h + 1], cp[:])
                    else:
                        nc.scalar.copy(d["u0"][:, h:h + 1], cp[:])
                return s_c

            def mk_pow(k):
                # level k: NT_k = NT_{k-1}^2 (pair), with the V-apply (k<=6)
                # or U-apply (k>=7) folded into the matmuls as extra moving
                # columns.  w = number of new V/U columns this level.
                w = 1 << ((k - 1) if k <= 6 else (k - 7))
                side = 0 if k <= 6 else 1   # which ring side carries cols

                def s_pow(t):
                    m, h = t // 2, t % 2
                    d = P[m]
                    Rp = d[f"NT{k - 1}"]
                    if h == 0:
                        d[f"p{k}"] = PG()
                    p = d[f"p{k}"]
                    if k <= 6:
                        # [N_k | dA^{2^(k-1)} Vold]
                        MM(p[:, 0, h, 0:128 + w], Rp[:, 1, h, 0:128],
                           Rp[:, 0, h, 0:128 + w])
                        MM(p[:, 1, h, 0:128 + w], Rp[:, 0, h, 0:128],
                           Rp[:, 1, h, 0:128 + w])
                    elif k <= 10:
                        MM(p[:, 0, h, 0:128 + w], Rp[:, 1, h, 0:128],
                           Rp[:, 0, h, 0:128 + w])
                        # [T_k | M^{2^(k-7)T} Uold]
                        MM(p[:, 1, h, 0:128 + w], Rp[:, 0, h, 0:128],
                           Rp[:, 1, h, 0:128 + w])
                    else:  # k == 11: N-only squaring + U-apply j=4
                        MM(p[:, 0, h, 0:128], Rp[:, 1, h, 0:128],
                           Rp[:, 0, h, 0:128])
                        MM(p[:, 1, h, 0:w], Rp[:, 0, h, 0:128],
                           Rp[:, 1, h, 128:128 + w])
                    if h == 1:
                        R = RING()
                        if k <= 10:
                            ev(R[:, :, :, 0:128 + w], p[:, :, :, 0:128 + w])
                        else:
                            ev(R[:, 0, :, 0:128], p[:, 0, :, 0:128])
                            ev(R[:, 1, :, 128:128 + w], p[:, 1, :, 0:w])
                        # carry the old V/U columns (newest-first order);
                        # both sides so padded streams stay initialized
                        nc.gpsimd.tensor_copy(
                            R[:, :, :, 128 + w:128 + 2 * w],
                            Rp[:, :, :, 128:128 + w])
                        if k == 6:
                            # V complete: extract to standalone tile
                            Vf = sb.tile([128, 2, 64], f16, tag="Vf",
                                         name="Vf", bufs=npair + 1)
                            nc.gpsimd.tensor_copy(Vf[:], R[:, 0, :, 128:192])
                            d["Vfin"] = Vf
                            # inject U0 at U-area col 0 for the k=7 merge
                            nc.gpsimd.tensor_copy(R[:, 1, 0, 128:129],
                                                  d["u0"][:, 0:1])
                            nc.gpsimd.tensor_copy(R[:, 1, 1, 128:129],
                                                  d["u0"][:, 1:2])
                        d[f"NT{k}"] = R
                return s_pow

            def s_u32(t):
                # U-apply j=5: U32 = NT11^T Uold(32)
                m, h = t // 2, t % 2
                d = P[m]
                R = d["NT11"]
                if h == 0:
                    d["Ufin"] = sb.tile([128, 2, 64], f16, tag="Uf",
                                        name="Uf", bufs=npair + 1)
                up = SMM(R[:, 0, h, 0:128], R[:, 1, h, 128:160], w=32)
                sm(d["Ufin"][:, h, 0:32], up[:])
                if h == 1:
                    nc.gpsimd.tensor_copy(d["Ufin"][:, :, 32:64],
                                          R[:, 1, :, 128:160])

            def s_fin(t):
                m, h = t // 2, t % 2
                d = P[m]
                # two separate "sm" banks: the two matmuls use different PE
                # row groups and run concurrently -- same-bank PSUM writes
                # from concurrent row groups are a fatal HW collision
                pKa = ps.tile([64, 64], f32, tag="sm", bufs=2, name="pKa")
                pKb = ps.tile([64, 64], f32, tag="sm", bufs=2, name="pKb")
                nc.tensor.matmul(pKa[:], d["Ufin"][0:64, h, :],
                                 d["Vfin"][0:64, h, :], start=True, stop=True)
                nc.tensor.matmul(pKb[:], d["Ufin"][64:128, h, :],
                                 d["Vfin"][64:128, h, :], start=True,
                                 stop=True)
                sm(OUT[:, t * 128:t * 128 + 64], pKa[:])
                sm(OUT[:, t * 128 + 64:(t + 1) * 128], pKb[:])

            stages = ([s_f0, s_e2, s_f1, s_e4, s_e8] +
                      [mk_c(0), mk_c(1), mk_c(2), mk_c(3)] +
                      [s_pp, s_nt0] +
                      [mk_pow(k) for k in range(1, 12)] +
                      [s_u32, s_fin])
            ns = len(stages)
            # skewed (wavefront) emission: tile t runs stage s at step s + t.
            # Descending tile order within a step so pair-shared tensors
            # produced by the odd tile (stage s) precede the even tile's
            # stage s+1 in the same step.
            for step in range(ns + nt - 1):
                for t in reversed(range(nt)):
                    s = step - t
                    if 0 <= s < ns:
                        stages[s](t)

            # output DMAs (2 chunks)
            half = nt // 2 * 128
            nc.scalar.dma_start(Y[:, 0:half], OUT[:, 0:half])
            nc.sync.dma_start(Y[:, half:2 * half], OUT[:, half:2 * half])
            rep.__exit__(None, None, None)

    nc.compile()
    return nc


def _host_pack(A, B, C, log_dt):
    A = np.asarray(A, np.float32)
    B = np.asarray(B, np.float32)
    C = np.asarray(C, np.float32)
    log_dt = np.asarray(log_dt, np.float32)
    dt = np.exp(log_dt)
    E = (0.5 * dt)[:, None, None].astype(np.float32) * A      # [H,64,64]
    ETc = np.swapaxes(E, 1, 2)
    dtB = (dt[:, None] * B).astype(np.float32)

    G = A.shape[0] // 2   # number of block-diag tiles, all cores
    I64 = np.eye(64, dtype=np.float32)

    def blockdiag(M):  # [H,64,64] -> [G,128,128] fp16
        out = np.zeros((G, 128, 128), np.float16)
        out[:, 0:64, 0:64] = M[0::2]
        out[:, 64:128, 64:128] = M[1::2]
        return out

    Epk = blockdiag(E)
    ETpk = blockdiag(ETc)
    H0pk = blockdiag(2.0 * E + I64)
    H0Tpk = blockdiag(2.0 * ETc + I64)
    # IN layout per tile: [E | ET | H0 | H0T] -> [G, 128, 512]
    INpk = np.concatenate([Epk, ETpk, H0pk, H0Tpk], axis=2)
    BCt = np.zeros((G, 128, 2), np.float16)   # per tile [dtB | C]
    BCt[:, 0:64, 0] = dtB[0::2]
    BCt[:, 64:128, 0] = dtB[1::2]
    BCt[:, 0:64, 1] = C[0::2]
    BCt[:, 64:128, 1] = C[1::2]
    # per pair: [dtB_a | dtB_b | C_a | C_b]
    BCpk = np.zeros((G // 2, 128, 4), np.float16)
    BCpk[:, :, 0] = BCt[0::2, :, 0]
    BCpk[:, :, 1] = BCt[1::2, :, 0]
    BCpk[:, :, 2] = BCt[0::2, :, 1]
    BCpk[:, :, 3] = BCt[1::2, :, 1]
    I128 = np.eye(128, dtype=np.float16)
    IWpk = np.concatenate([I128, I128], axis=1)   # [128, 256]
    return INpk, BCpk, IWpk


def _in_maps(A, B, C, log_dt):
    INpk, BCpk, IWpk = _host_pack(A, B, C, log_dt)
    maps = []
    for c in range(NCORES):
        s = slice(c * NT, (c + 1) * NT)
        INc = np.ascontiguousarray(
            INpk[s].transpose(1, 0, 2).reshape(128, NT * 512))
        sp = slice(c * NP, (c + 1) * NP)
        BCc = np.ascontiguousarray(
            BCpk[sp].transpose(1, 0, 2).reshape(128, NT * 2))
        maps.append({"INd": INc, "BCd": BCc, "IWd": IWpk})
    return maps


def kernel(A, B, C, log_dt, L):
    from concourse.bass_utils import run_bass_kernel_spmd

    if "nc" not in _cache:
        _cache["nc"] = _build_program(NT)
    nc = _cache["nc"]

    res = run_bass_kernel_spmd(nc, _in_maps(A, B, C, log_dt),
                               core_ids=list(range(NCORES)))
    K = np.empty((H, L), np.float32)
    for c in range(NCORES):
        Yc = res.results[c]["Y"]            # [64, NT*128]
        for t in range(NT):
            blk = Yc[:, t * 128:(t + 1) * 128]
            # U/V columns are stored newest-first => both axes reversed
            K[c * CH_PER_CORE + 2 * t] = blk[::-1, 0:64][:, ::-1].reshape(L)
            K[c * CH_PER_CORE + 2 * t + 1] = (
                blk[::-1, 64:128][:, ::-1].reshape(L))
    return K
